# revision 3
# baseline (speedup 1.0000x reference)
"""Trainium2 Bass kernel for nn_MultiHeadAttention_50534585205084 (sparse pooled attention).

Sharding (8 cores): batch (4) x head-half (2). Core c handles batch c//2's
heads [8*(c%2), 8*(c%2)+8) via column-sharded Wq/Wk/Wv and row-sharded Wc.
Each core emits a PARTIAL final projection yT [1024, 256] (pooled rows,
transposed); the host sums the two halves per batch, upsamples rows 8x
(the reference's repeat+crop makes the final output row-periodic with
period KP=8: every op after the pooled attention is position-wise), and
adds bc.

On-chip math (per core), all matmuls bf16 with fp32 PSUM accumulation:
  phase A: for each of q/k/v: xT[1024,2048] @ W -> channel-major conv input
           [512 ch, 2048 seq]; causal depthwise conv (DK=3) fused with causal
           avg-pool (KP=8) as 3 shifted grouped-sum reductions combined with
           per-channel weights (pool's 1/KP and the DD**-0.25 norm are folded
           into host-side weights); all dense/conv biases folded in exactly
           (incl. the i=0 partial-window correction).
  phase B: per head: transposed logits E_T[m,n]=exp(qp.kp) (no max-sub needed:
           |logits|<<1 by construction), causal mask as elementwise 0/1
           multiply on the two diagonal blocks (the all-masked block is
           skipped), softmax denominator via ones-matmul, unnormalized
           out_T = vp_m @ E_T, normalized with a partition-broadcast
           reciprocal, then the shared head up-projection Wup.
  phase C: merged [512, 256] @ row-shard of Wc -> yT [1024, 256].
"""
import sys
sys.path.insert(0, '/opt/trn_rl_repo')

from contextlib import ExitStack

import numpy as np
import ml_dtypes

import concourse.bass as bass
import concourse.mybir as mybir
import concourse.tile as tile
from concourse import bacc
from concourse.bass_utils import run_bass_kernel_spmd
from concourse.masks import make_identity

B, S, D, H, KP, DK = 4, 2048, 1024, 16, 8, 3
DD = D // H            # 64 head dim
N_CORES = 8
C = D // 2             # 512 channels per core (8 heads)
NP = S // KP           # 256 pooled positions
P = 128
NK = D // P            # 8 contraction tiles
NCT = C // P           # 4 channel tiles (2 heads each)
NSC = S // 512         # 4 seq chunks in phase A
NORM = float(DD) ** -0.25

dt = mybir.dt
AF = mybir.ActivationFunctionType
OP = mybir.AluOpType


def _emit(nc, tc, aps):
    qT, kT, vT = aps["qT"], aps["kT"], aps["vT"]
    wq, wk, wv = aps["wq"], aps["wk"], aps["wv"]
    wc, wup, mask, biasw, bup2, yT = (
        aps["wc"], aps["wup"], aps["mask"], aps["biasw"], aps["bup2"], aps["yT"])

    with ExitStack() as ctx:
        wpool = ctx.enter_context(tc.tile_pool(name="w", bufs=1))
        xpool = ctx.enter_context(tc.tile_pool(name="x", bufs=2))
        spool = ctx.enter_context(tc.tile_pool(name="s", bufs=3))
        rpool = ctx.enter_context(tc.tile_pool(name="r", bufs=3))
        ppool = ctx.enter_context(tc.tile_pool(name="p", bufs=1))
        apool = ctx.enter_context(tc.tile_pool(name="a", bufs=2))
        ypool = ctx.enter_context(tc.tile_pool(name="y", bufs=2))
        psum = ctx.enter_context(tc.tile_pool(name="ps", bufs=8, space="PSUM"))

        # --- persistent constants/weights ---
        w_sbs = {}
        for nm, ap in (("q", wq), ("k", wk), ("v", wv)):
            t = wpool.tile([P, NK, C], dt.bfloat16, tag=f"w{nm}")
            apr = ap.rearrange("(k p) c -> p k c", p=P)
            for k in range(NK):
                nc.sync.dma_start(t[:, k, :], apr[:, k, :])
            w_sbs[nm] = t
        wc_sb = wpool.tile([P, NCT, D], dt.bfloat16, tag="wc")
        wcr = wc.rearrange("(t p) d -> p t d", p=P)
        for t_ in range(NCT):
            nc.sync.dma_start(wc_sb[:, t_, :], wcr[:, t_, :])
        wup_sb = wpool.tile([DD, DD], dt.bfloat16, tag="wup")
        nc.sync.dma_start(wup_sb[:], wup[:])
        mask_sb = wpool.tile([P, P], dt.bfloat16, tag="mask")
        nc.sync.dma_start(mask_sb[:], mask[:])
        biasw_sb = wpool.tile([P, NCT, 3, 6], dt.float32, tag="biasw")
        nc.sync.dma_start(biasw_sb[:], biasw.rearrange("p (t j s) -> p t j s", t=NCT, j=3))
        bup2_sb = wpool.tile([P, 1], dt.float32, tag="bup2")
        nc.sync.dma_start(bup2_sb[:], bup2[:])
        ones_sb = wpool.tile([P, 1], dt.bfloat16, tag="ones")
        nc.vector.memset(ones_sb[:], 1.0)
        ident_sb = wpool.tile([P, P], dt.bfloat16, tag="ident")
        make_identity(nc, ident_sb[:])

        def BW(ct, pj, col):
            return biasw_sb[:, ct, pj, col:col + 1]

        # --- phase A: projections + causal depthwise conv + causal avg pool ---
        pooled = {}
        for pj, (nm, x_ap) in enumerate((("q", qT), ("k", kT), ("v", vT))):
            xT_sb = xpool.tile([P, NK, S], dt.bfloat16, tag="xT")
            xr = x_ap.rearrange("(k p) s -> p k s", p=P)
            for k in range(NK):
                nc.sync.dma_start(xT_sb[:, k, :], xr[:, k, :])
            w_sb = w_sbs[nm]
            pl = ppool.tile([P, NCT, NP], dt.bfloat16, tag=f"pool_{nm}")
            pooled[nm] = pl
            for ct in range(NCT):
                xs = spool.tile([P, KP + 1 + S], dt.bfloat16, tag="xs")
                nc.vector.memset(xs[:, 0:KP + 1], 0.0)
                for sc in range(NSC):
                    ps = psum.tile([P, 512], dt.float32, tag="ps")
                    for k in range(NK):
                        nc.tensor.matmul(
                            ps[:], w_sb[:, k, ct * P:(ct + 1) * P],
                            xT_sb[:, k, sc * 512:(sc + 1) * 512],
                            start=(k == 0), stop=(k == NK - 1))
                    nc.scalar.activation(
                        xs[:, KP + 1 + sc * 512: KP + 1 + (sc + 1) * 512], ps[:],
                        AF.Identity, bias=BW(ct, pj, 3), scale=1.0)
                # 3 shifted pooled sums; window t covers conv tap t
                prs = []
                for t_ in range(DK):
                    pr = rpool.tile([P, NP], dt.float32, tag=f"pr{t_}")
                    nc.vector.tensor_reduce(
                        pr[:], xs[:, t_:t_ + S].rearrange("p (n w) -> p n w", w=KP),
                        axis=mybir.AxisListType.X, op=OP.add)
                    prs.append(pr)
                tmp = rpool.tile([P, NP], dt.float32, tag="tmpc")
                nc.vector.tensor_scalar(
                    tmp[:], prs[0][:], BW(ct, pj, 0), BW(ct, pj, 4),
                    op0=OP.mult, op1=OP.add)
                nc.vector.scalar_tensor_tensor(
                    tmp[:], prs[1][:], BW(ct, pj, 1), tmp[:],
                    op0=OP.mult, op1=OP.add)
                nc.vector.scalar_tensor_tensor(
                    pl[:, ct, :], prs[2][:], BW(ct, pj, 2), tmp[:],
                    op0=OP.mult, op1=OP.add)
                # first pooled window only sees conv output 0: fix its bias
                nc.vector.tensor_scalar_add(
                    pl[:, ct, 0:1], pl[:, ct, 0:1], BW(ct, pj, 5))

        # --- phase B prep: vp into [m, c] layout via PE transpose ---
        vpm = [ppool.tile([P, NCT, P], dt.bfloat16, tag=f"vpm{mb}", name=f"vpm{mb}")
               for mb in range(2)]
        for ct in range(NCT):
            for mb in range(2):
                pst = psum.tile([P, P], dt.bfloat16, tag="ps")
                nc.tensor.transpose(
                    pst[:], pooled["v"][:, ct, mb * P:(mb + 1) * P], ident_sb[:])
                nc.vector.tensor_copy(vpm[mb][:, ct, :], pst[:])

        # --- phase B: per-head pooled causal attention (transposed layout) ---
        merged = ppool.tile([P, NCT, NP], dt.bfloat16, tag="merged")
        for h in range(H // 2):
            ct, half = h // 2, h % 2
            rows = slice(DD * half, DD * half + DD)
            qp_h = pooled["q"][rows, ct, :]
            kp_h = pooled["k"][rows, ct, :]
            # E_T[m, n] = exp(qp[n] . kp[m]); block (m1, n0) fully masked -> skipped
            psS0 = psum.tile([P, NP], dt.float32, tag="ps")
            nc.tensor.matmul(psS0[:], kp_h[:, 0:P], qp_h[:, :], start=True, stop=True)
            psS1 = psum.tile([P, P], dt.float32, tag="ps")
            nc.tensor.matmul(psS1[:], kp_h[:, P:NP], qp_h[:, P:NP], start=True, stop=True)
            E0 = apool.tile([P, NP], dt.bfloat16, tag="E0")
            nc.scalar.activation(E0[:], psS0[:], AF.Exp)
            E1 = apool.tile([P, P], dt.bfloat16, tag="E1")
            nc.scalar.activation(E1[:], psS1[:], AF.Exp)
            nc.vector.tensor_mul(E0[:, 0:P], E0[:, 0:P], mask_sb[:])
            nc.vector.tensor_mul(E1[:], E1[:], mask_sb[:])
            # softmax denominator: column sums of E_T via ones-matmul
            psSum = psum.tile([1, NP], dt.float32, tag="ps")
            nc.tensor.matmul(psSum[:, :], ones_sb[:], E0[:], start=True, stop=False)
            nc.tensor.matmul(psSum[:, P:NP], ones_sb[:], E1[:], start=False, stop=True)
            recip = apool.tile([1, NP], dt.float32, tag="recip")
            nc.vector.reciprocal(recip[:], psSum[:])
            rb = apool.tile([DD, NP], dt.float32, tag="rb")
            nc.gpsimd.partition_broadcast(rb[:], recip[:])
            # unnormalized out_T[dd, n] = sum_m vp[m, dd] E_T[m, n]
            psU = psum.tile([DD, NP], dt.float32, tag="ps")
            nc.tensor.matmul(psU[:], vpm[0][:, ct, rows], E0[:], start=True, stop=False)
            nc.tensor.matmul(psU[:, P:NP], vpm[1][:, ct, rows], E1[:], start=False, stop=True)
            outT = apool.tile([DD, NP], dt.bfloat16, tag="outT")
            nc.vector.tensor_mul(outT[:], psU[:], rb[:])
            # shared up-projection: up2_T = Wup.T @ out_T + bup
            psP = psum.tile([DD, NP], dt.float32, tag="ps")
            nc.tensor.matmul(psP[:], wup_sb[:], outT[:], start=True, stop=True)
            nc.scalar.activation(
                merged[rows, ct, :], psP[:], AF.Identity,
                bias=bup2_sb[rows, :], scale=1.0)

        # --- phase C: yT = Wc_half.T-partial @ merged ---
        for dti in range(D // P):
            psY = psum.tile([P, NP], dt.float32, tag="ps")
            for ct in range(NCT):
                nc.tensor.matmul(
                    psY[:], wc_sb[:, ct, dti * P:(dti + 1) * P], merged[:, ct, :],
                    start=(ct == 0), stop=(ct == NCT - 1))
            ysb = ypool.tile([P, NP], dt.float32, tag="y")
            nc.scalar.copy(ysb[:], psY[:])
            nc.sync.dma_start(yT[dti * P:(dti + 1) * P, :], ysb[:])


def build():
    nc = bacc.Bacc("TRN2", target_bir_lowering=False, debug=False,
                   num_devices=N_CORES)
    aps = {}
    for nm in ("qT", "kT", "vT"):
        aps[nm] = nc.dram_tensor(nm, [D, S], dt.bfloat16, kind="ExternalInput").ap()
    for nm in ("wq", "wk", "wv"):
        aps[nm] = nc.dram_tensor(nm, [D, C], dt.bfloat16, kind="ExternalInput").ap()
    aps["wc"] = nc.dram_tensor("wc", [C, D], dt.bfloat16, kind="ExternalInput").ap()
    aps["wup"] = nc.dram_tensor("wup", [DD, DD], dt.bfloat16, kind="ExternalInput").ap()
    aps["mask"] = nc.dram_tensor("mask", [P, P], dt.bfloat16, kind="ExternalInput").ap()
    aps["biasw"] = nc.dram_tensor("biasw", [P, NCT * 3 * 6], dt.float32,
                                  kind="ExternalInput").ap()
    aps["bup2"] = nc.dram_tensor("bup2", [P, 1], dt.float32, kind="ExternalInput").ap()
    aps["yT"] = nc.dram_tensor("yT", [D, NP], dt.float32, kind="ExternalOutput").ap()
    with tile.TileContext(nc) as tc:
        _emit(nc, tc, aps)
    nc.compile()
    return nc


_BUILT = None


def _get_built():
    global _BUILT
    if _BUILT is None:
        _BUILT = build()
    return _BUILT


def make_in_maps(q, k, v, Wq, bq, Wk, bk, Wv, bv, Wup, bup, Wc, bc,
                 wcq, bcq, wck, bck, wcv, bcv):
    bf = ml_dtypes.bfloat16
    q, k, v = (np.asarray(x, np.float32) for x in (q, k, v))
    mask_np = np.triu(np.ones((P, P), np.float32)).astype(bf)
    in_maps = []
    for core in range(N_CORES):
        b, half = core // 2, core % 2
        cs = slice(half * C, half * C + C)
        biasw = np.zeros((P, NCT, 3, 6), np.float32)
        for ct in range(NCT):
            ch = slice(half * C + ct * P, half * C + (ct + 1) * P)
            for pj, (cw, cb, db, scale) in enumerate((
                    (wcq, bcq, bq, NORM), (wck, bck, bk, NORM), (wcv, bcv, bv, 1.0))):
                cw = np.asarray(cw, np.float32)
                biasw[:, ct, pj, 0:3] = (cw[:, ch] / KP).T
                biasw[:, ct, pj, 3] = np.asarray(db, np.float32)[ch] * scale
                biasw[:, ct, pj, 4] = np.asarray(cb, np.float32)[ch]
                biasw[:, ct, pj, 5] = -(KP - 1) / KP * np.asarray(cb, np.float32)[ch]
        in_maps.append({
            "qT": np.ascontiguousarray(q[b].T).astype(bf),
            "kT": np.ascontiguousarray(k[b].T).astype(bf),
            "vT": np.ascontiguousarray(v[b].T).astype(bf),
            "wq": (np.asarray(Wq, np.float32)[:, cs] * NORM).astype(bf),
            "wk": (np.asarray(Wk, np.float32)[:, cs] * NORM).astype(bf),
            "wv": np.asarray(Wv, np.float32)[:, cs].astype(bf),
            "wc": np.asarray(Wc, np.float32)[cs, :].astype(bf),
            "wup": np.asarray(Wup, np.float32).astype(bf),
            "mask": mask_np,
            "biasw": biasw.reshape(P, NCT * 3 * 6),
            "bup2": np.tile(np.asarray(bup, np.float32), 2).reshape(P, 1),
        })
    return in_maps


def gather(results, bc):
    out = np.empty((B, S, D), np.float32)
    for b in range(B):
        y = results[2 * b]["yT"] + results[2 * b + 1]["yT"]   # [D, NP]
        out[b] = np.repeat(y.T, KP, axis=0) + np.asarray(bc, np.float32)[None, :]
    return out


def kernel(q, k, v, Wq, bq, Wk, bk, Wv, bv, Wup, bup, Wc, bc,
           wcq, bcq, wck, bck, wcv, bcv):
    nc = _get_built()
    in_maps = make_in_maps(q, k, v, Wq, bq, Wk, bk, Wv, bv, Wup, bup, Wc, bc,
                           wcq, bcq, wck, bck, wcv, bcv)
    res = run_bass_kernel_spmd(nc, in_maps, core_ids=list(range(N_CORES)),
                               trace=False)
    return gather(res.results, bc)


# revision 8
# speedup vs baseline: 1.0942x; 1.0942x over previous
"""Trainium2 Bass kernel for nn_MultiHeadAttention_50534585205084 (sparse pooled attention).

Sharding (8 cores): batch (4) x head-half (2). Core c handles batch c//2's
heads [8*(c%2), 8*(c%2)+8) via column-sharded Wq/Wk/Wv and row-sharded Wc.
Each core emits a PARTIAL final projection yT [1024, 256] (pooled rows,
transposed); the host sums the two halves per batch, upsamples rows 8x
(the reference's repeat+crop makes the final output row-periodic with
period KP=8: every op after the pooled attention is position-wise), and
adds bc.

On-chip math (per core), all matmuls bf16 with fp32 PSUM accumulation:
  phase A: for each of q/k/v: xT[1024,2048] @ W -> channel-major conv input
           [512 ch, 2048 seq]; causal depthwise conv (DK=3) fused with causal
           avg-pool (KP=8) as 3 shifted grouped-sum reductions combined with
           per-channel weights (pool's 1/KP and the DD**-0.25 norm are folded
           into host-side weights); all dense/conv biases folded in exactly
           (incl. the i=0 partial-window correction).
  phase B: per head: transposed logits E_T[m,n]=exp(qp.kp) (no max-sub needed:
           |logits|<<1 by construction), causal mask as elementwise 0/1
           multiply on the two diagonal blocks (the all-masked block is
           skipped), softmax denominator via ones-matmul, unnormalized
           out_T = vp_m @ E_T, normalized with a partition-broadcast
           reciprocal, then the shared head up-projection Wup.
  phase C: merged [512, 256] @ row-shard of Wc -> yT [1024, 256].
"""
import sys
sys.path.insert(0, '/opt/trn_rl_repo')

from contextlib import ExitStack

import numpy as np
import ml_dtypes

import concourse.bass as bass
import concourse.mybir as mybir
import concourse.tile as tile
from concourse import bacc
from concourse.bass_utils import run_bass_kernel_spmd
from concourse.masks import make_identity

B, S, D, H, KP, DK = 4, 2048, 1024, 16, 8, 3
DD = D // H            # 64 head dim
N_CORES = 8
C = D // 2             # 512 channels per core (8 heads)
NP = S // KP           # 256 pooled positions
P = 128
NK = D // P            # 8 contraction tiles
NCT = C // P           # 4 channel tiles (2 heads each)
NSC = S // 512         # 4 seq chunks in phase A
NORM = float(DD) ** -0.25

dt = mybir.dt
AF = mybir.ActivationFunctionType
OP = mybir.AluOpType


def _emit(nc, tc, aps):
    qT, kT, vT = aps["qT"], aps["kT"], aps["vT"]
    wq, wk, wv = aps["wq"], aps["wk"], aps["wv"]
    wc, wup, mask, biasw, bup2, yT = (
        aps["wc"], aps["wup"], aps["mask"], aps["biasw"], aps["bup2"], aps["yT"])

    with ExitStack() as ctx:
        wpool = ctx.enter_context(tc.tile_pool(name="w", bufs=1))
        xpool = ctx.enter_context(tc.tile_pool(name="x", bufs=2))
        spool = ctx.enter_context(tc.tile_pool(name="s", bufs=3))
        rpool = ctx.enter_context(tc.tile_pool(name="r", bufs=3))
        ppool = ctx.enter_context(tc.tile_pool(name="p", bufs=1))
        apool = ctx.enter_context(tc.tile_pool(name="a", bufs=2))
        ypool = ctx.enter_context(tc.tile_pool(name="y", bufs=2))
        psum = ctx.enter_context(tc.tile_pool(name="ps", bufs=8, space="PSUM"))

        # --- persistent constants/weights ---
        w_sbs = {}
        for nm, ap in (("q", wq), ("k", wk), ("v", wv)):
            t = wpool.tile([P, NK, C], dt.bfloat16, tag=f"w{nm}")
            apr = ap.rearrange("(k p) c -> p k c", p=P)
            for k in range(NK):
                nc.sync.dma_start(t[:, k, :], apr[:, k, :])
            w_sbs[nm] = t
        wc_sb = wpool.tile([P, NCT, D], dt.bfloat16, tag="wc")
        wcr = wc.rearrange("(t p) d -> p t d", p=P)
        for t_ in range(NCT):
            nc.sync.dma_start(wc_sb[:, t_, :], wcr[:, t_, :])
        wup_sb = wpool.tile([DD, DD], dt.bfloat16, tag="wup")
        nc.sync.dma_start(wup_sb[:], wup[:])
        mask_sb = wpool.tile([P, P], dt.bfloat16, tag="mask")
        nc.sync.dma_start(mask_sb[:], mask[:])
        biasw_sb = wpool.tile([P, NCT, 3, 8], dt.float32, tag="biasw")
        nc.sync.dma_start(biasw_sb[:], biasw.rearrange("p (t j s) -> p t j s", t=NCT, j=3))
        bup2_sb = wpool.tile([P, 1], dt.float32, tag="bup2")
        nc.sync.dma_start(bup2_sb[:], bup2[:])
        ones_sb = wpool.tile([P, 1], dt.bfloat16, tag="ones")
        nc.vector.memset(ones_sb[:], 1.0)
        ident_sb = wpool.tile([P, P], dt.bfloat16, tag="ident")
        make_identity(nc, ident_sb[:])

        def BW(ct, pj, col):
            return biasw_sb[:, ct, pj, col:col + 1]

        # 3 rotating conv/pool staging buffers; zero pads written once
        xs_tiles = [wpool.tile([P, KP + 1 + S], dt.bfloat16, tag=f"xs{i}",
                               name=f"xs{i}") for i in range(3)]
        for t in xs_tiles:
            nc.vector.memset(t[:, 0:KP + 1], 0.0)

        # --- phase A: projections + causal depthwise conv + causal avg pool.
        # conv taps folded into ONE 8-wide pooled sum (ps2) plus strided
        # edge corrections:
        #   pooled = A*ps2 - B*x[8i] - C*x[8i-1] + B*x[8i-8] + C*x[8i-9] + bcv
        # with A=(w0+w1+w2)/8, B=(w0+w1)/8, C=w0/8 per channel.
        pooled = {}
        for pj, (nm, x_ap) in enumerate((("q", qT), ("k", kT), ("v", vT))):
            xT_sb = xpool.tile([P, NK, S], dt.bfloat16, tag="xT")
            xr = x_ap.rearrange("(k p) s -> p k s", p=P)
            for k in range(NK):
                nc.sync.dma_start(xT_sb[:, k, :], xr[:, k, :])
            w_sb = w_sbs[nm]
            pl = ppool.tile([P, NCT, NP], dt.bfloat16, tag=f"pool_{nm}")
            pooled[nm] = pl
            for ct in range(NCT):
                xs = xs_tiles[(pj * NCT + ct) % 3]
                for sc in range(NSC):
                    ps = psum.tile([P, 512], dt.float32, tag="ps")
                    for k in range(NK):
                        nc.tensor.matmul(
                            ps[:], w_sb[:, k, ct * P:(ct + 1) * P],
                            xT_sb[:, k, sc * 512:(sc + 1) * 512],
                            start=(k == 0), stop=(k == NK - 1))
                    nc.scalar.activation(
                        xs[:, KP + 1 + sc * 512: KP + 1 + (sc + 1) * 512], ps[:],
                        AF.Identity, bias=BW(ct, pj, 5), scale=1.0)

                def col(off):  # [256] strided-by-8 view starting at buffer col off
                    return xs[:, off:off + S].rearrange("p (n w) -> p n w", w=KP)[:, :, 0]

                r = rpool.tile([P, NP], dt.float32, tag="ps2")
                nc.vector.tensor_reduce(
                    r[:], xs[:, 2:2 + S].rearrange("p (n w) -> p n w", w=KP),
                    axis=mybir.AxisListType.X, op=OP.add)
                tmp = rpool.tile([P, NP], dt.float32, tag="tmpc")
                nc.vector.tensor_scalar(
                    tmp[:], r[:], BW(ct, pj, 0), BW(ct, pj, 6),
                    op0=OP.mult, op1=OP.add)
                for coli, xoff in ((1, KP + 1), (2, KP), (3, 1)):
                    nc.vector.scalar_tensor_tensor(
                        tmp[:], col(xoff), BW(ct, pj, coli), tmp[:],
                        op0=OP.mult, op1=OP.add)
                nc.vector.scalar_tensor_tensor(
                    pl[:, ct, :], col(0), BW(ct, pj, 4), tmp[:],
                    op0=OP.mult, op1=OP.add)
                # first pooled window only sees conv output 0: fix its bias
                nc.vector.tensor_scalar_add(
                    pl[:, ct, 0:1], pl[:, ct, 0:1], BW(ct, pj, 7))

        # --- phase B prep: vp into [m, c] layout via PE transpose ---
        vpm = [ppool.tile([P, NCT, P], dt.bfloat16, tag=f"vpm{mb}", name=f"vpm{mb}")
               for mb in range(2)]
        for ct in range(NCT):
            for mb in range(2):
                pst = psum.tile([P, P], dt.bfloat16, tag="ps")
                nc.tensor.transpose(
                    pst[:], pooled["v"][:, ct, mb * P:(mb + 1) * P], ident_sb[:])
                nc.vector.tensor_copy(vpm[mb][:, ct, :], pst[:])

        # --- phase B: per-head pooled causal attention (transposed layout) ---
        merged = ppool.tile([P, NCT, NP], dt.bfloat16, tag="merged")
        for h in range(H // 2):
            ct, half = h // 2, h % 2
            rows = slice(DD * half, DD * half + DD)
            qp_h = pooled["q"][rows, ct, :]
            kp_h = pooled["k"][rows, ct, :]
            # E_T[m, n] = exp(qp[n] . kp[m]); block (m1, n0) fully masked -> skipped
            psS0 = psum.tile([P, NP], dt.float32, tag="ps")
            nc.tensor.matmul(psS0[:], kp_h[:, 0:P], qp_h[:, :], start=True, stop=True)
            psS1 = psum.tile([P, P], dt.float32, tag="ps")
            nc.tensor.matmul(psS1[:], kp_h[:, P:NP], qp_h[:, P:NP], start=True, stop=True)
            E0 = apool.tile([P, NP], dt.bfloat16, tag="E0")
            nc.scalar.activation(E0[:], psS0[:], AF.Exp)
            E1 = apool.tile([P, P], dt.bfloat16, tag="E1")
            nc.scalar.activation(E1[:], psS1[:], AF.Exp)
            nc.vector.tensor_mul(E0[:, 0:P], E0[:, 0:P], mask_sb[:])
            nc.vector.tensor_mul(E1[:], E1[:], mask_sb[:])
            # softmax denominator: column sums of E_T via ones-matmul
            psSum = psum.tile([1, NP], dt.float32, tag="ps")
            nc.tensor.matmul(psSum[:, :], ones_sb[:], E0[:], start=True, stop=False)
            nc.tensor.matmul(psSum[:, P:NP], ones_sb[:], E1[:], start=False, stop=True)
            recip = apool.tile([1, NP], dt.float32, tag="recip")
            nc.vector.reciprocal(recip[:], psSum[:])
            rb = apool.tile([DD, NP], dt.float32, tag="rb")
            nc.gpsimd.partition_broadcast(rb[:], recip[:])
            # unnormalized out_T[dd, n] = sum_m vp[m, dd] E_T[m, n]
            psU = psum.tile([DD, NP], dt.float32, tag="ps")
            nc.tensor.matmul(psU[:], vpm[0][:, ct, rows], E0[:], start=True, stop=False)
            nc.tensor.matmul(psU[:, P:NP], vpm[1][:, ct, rows], E1[:], start=False, stop=True)
            outT = apool.tile([DD, NP], dt.bfloat16, tag="outT")
            nc.vector.tensor_mul(outT[:], psU[:], rb[:])
            # shared up-projection: up2_T = Wup.T @ out_T + bup
            psP = psum.tile([DD, NP], dt.float32, tag="ps")
            nc.tensor.matmul(psP[:], wup_sb[:], outT[:], start=True, stop=True)
            nc.scalar.activation(
                merged[rows, ct, :], psP[:], AF.Identity,
                bias=bup2_sb[rows, :], scale=1.0)

        # --- phase C: yT = Wc_half.T-partial @ merged ---
        for dti in range(D // P):
            psY = psum.tile([P, NP], dt.float32, tag="ps")
            for ct in range(NCT):
                nc.tensor.matmul(
                    psY[:], wc_sb[:, ct, dti * P:(dti + 1) * P], merged[:, ct, :],
                    start=(ct == 0), stop=(ct == NCT - 1))
            ysb = ypool.tile([P, NP], dt.float32, tag="y")
            nc.scalar.copy(ysb[:], psY[:])
            nc.sync.dma_start(yT[dti * P:(dti + 1) * P, :], ysb[:])


def build():
    nc = bacc.Bacc("TRN2", target_bir_lowering=False, debug=False,
                   num_devices=N_CORES)
    aps = {}
    for nm in ("qT", "kT", "vT"):
        aps[nm] = nc.dram_tensor(nm, [D, S], dt.bfloat16, kind="ExternalInput").ap()
    for nm in ("wq", "wk", "wv"):
        aps[nm] = nc.dram_tensor(nm, [D, C], dt.bfloat16, kind="ExternalInput").ap()
    aps["wc"] = nc.dram_tensor("wc", [C, D], dt.bfloat16, kind="ExternalInput").ap()
    aps["wup"] = nc.dram_tensor("wup", [DD, DD], dt.bfloat16, kind="ExternalInput").ap()
    aps["mask"] = nc.dram_tensor("mask", [P, P], dt.bfloat16, kind="ExternalInput").ap()
    aps["biasw"] = nc.dram_tensor("biasw", [P, NCT * 3 * 8], dt.float32,
                                  kind="ExternalInput").ap()
    aps["bup2"] = nc.dram_tensor("bup2", [P, 1], dt.float32, kind="ExternalInput").ap()
    aps["yT"] = nc.dram_tensor("yT", [D, NP], dt.float32, kind="ExternalOutput").ap()
    with tile.TileContext(nc) as tc:
        _emit(nc, tc, aps)
    nc.compile()
    return nc


_BUILT = None


def _get_built():
    global _BUILT
    if _BUILT is None:
        _BUILT = build()
    return _BUILT


def make_in_maps(q, k, v, Wq, bq, Wk, bk, Wv, bv, Wup, bup, Wc, bc,
                 wcq, bcq, wck, bck, wcv, bcv):
    bf = ml_dtypes.bfloat16
    q, k, v = (np.asarray(x, np.float32) for x in (q, k, v))
    mask_np = np.triu(np.ones((P, P), np.float32)).astype(bf)
    in_maps = []
    for core in range(N_CORES):
        b, half = core // 2, core % 2
        cs = slice(half * C, half * C + C)
        biasw = np.zeros((P, NCT, 3, 8), np.float32)
        for ct in range(NCT):
            ch = slice(half * C + ct * P, half * C + (ct + 1) * P)
            for pj, (cw, cb, db, scale) in enumerate((
                    (wcq, bcq, bq, NORM), (wck, bck, bk, NORM), (wcv, bcv, bv, 1.0))):
                w0, w1, w2 = (np.asarray(cw, np.float32)[:, ch] / KP)
                bconv = np.asarray(cb, np.float32)[ch]
                biasw[:, ct, pj, 0] = w0 + w1 + w2          # A
                biasw[:, ct, pj, 1] = -(w0 + w1)            # -B
                biasw[:, ct, pj, 2] = -w0                   # -C
                biasw[:, ct, pj, 3] = w0 + w1               # +B
                biasw[:, ct, pj, 4] = w0                    # +C
                biasw[:, ct, pj, 5] = np.asarray(db, np.float32)[ch] * scale
                biasw[:, ct, pj, 6] = bconv
                biasw[:, ct, pj, 7] = -(KP - 1) / KP * bconv
        in_maps.append({
            "qT": np.ascontiguousarray(q[b].T).astype(bf),
            "kT": np.ascontiguousarray(k[b].T).astype(bf),
            "vT": np.ascontiguousarray(v[b].T).astype(bf),
            "wq": (np.asarray(Wq, np.float32)[:, cs] * NORM).astype(bf),
            "wk": (np.asarray(Wk, np.float32)[:, cs] * NORM).astype(bf),
            "wv": np.asarray(Wv, np.float32)[:, cs].astype(bf),
            "wc": np.asarray(Wc, np.float32)[cs, :].astype(bf),
            "wup": np.asarray(Wup, np.float32).astype(bf),
            "mask": mask_np,
            "biasw": biasw.reshape(P, NCT * 3 * 8),
            "bup2": np.tile(np.asarray(bup, np.float32), 2).reshape(P, 1),
        })
    return in_maps


def gather(results, bc):
    out = np.empty((B, S, D), np.float32)
    for b in range(B):
        y = results[2 * b]["yT"] + results[2 * b + 1]["yT"]   # [D, NP]
        out[b] = np.repeat(y.T, KP, axis=0) + np.asarray(bc, np.float32)[None, :]
    return out


def kernel(q, k, v, Wq, bq, Wk, bk, Wv, bv, Wup, bup, Wc, bc,
           wcq, bcq, wck, bck, wcv, bcv):
    nc = _get_built()
    in_maps = make_in_maps(q, k, v, Wq, bq, Wk, bk, Wv, bv, Wup, bup, Wc, bc,
                           wcq, bcq, wck, bck, wcv, bcv)
    res = run_bass_kernel_spmd(nc, in_maps, core_ids=list(range(N_CORES)),
                               trace=False)
    return gather(res.results, bc)


# revision 11
# speedup vs baseline: 1.3824x; 1.2635x over previous
"""Trainium2 Bass kernel for nn_MultiHeadAttention_50534585205084 (sparse pooled attention).

Sharding (8 cores): batch (4) x head-half (2). Core c handles batch c//2's
heads [8*(c%2), 8*(c%2)+8) via column-sharded Wq/Wk/Wv and row-sharded Wc.
Each core emits a PARTIAL final projection yT [1024, 256] (pooled rows,
transposed); the host sums the two halves per batch, upsamples rows 8x
(the reference's repeat+crop makes the final output row-periodic with
period KP=8: every op after the pooled attention is position-wise), and
adds bc.

On-chip math (per core), all matmuls bf16 with fp32 PSUM accumulation:
  phase A: for each of q/k/v: xT[1024,2048] @ W -> channel-major conv input
           [512 ch, 2048 seq]; causal depthwise conv (DK=3) fused with causal
           avg-pool (KP=8) as 3 shifted grouped-sum reductions combined with
           per-channel weights (pool's 1/KP and the DD**-0.25 norm are folded
           into host-side weights); all dense/conv biases folded in exactly
           (incl. the i=0 partial-window correction).
  phase B: per head: transposed logits E_T[m,n]=exp(qp.kp) (no max-sub needed:
           |logits|<<1 by construction), causal mask as elementwise 0/1
           multiply on the two diagonal blocks (the all-masked block is
           skipped), softmax denominator via ones-matmul, unnormalized
           out_T = vp_m @ E_T, normalized with a partition-broadcast
           reciprocal, then the shared head up-projection Wup.
  phase C: merged [512, 256] @ row-shard of Wc -> yT [1024, 256].
"""
import sys
sys.path.insert(0, '/opt/trn_rl_repo')

from contextlib import ExitStack

import numpy as np
import ml_dtypes

import concourse.bass as bass
import concourse.mybir as mybir
import concourse.tile as tile
from concourse import bacc
from concourse.bass_utils import run_bass_kernel_spmd
from concourse.masks import make_identity

B, S, D, H, KP, DK = 4, 2048, 1024, 16, 8, 3
DD = D // H            # 64 head dim
N_CORES = 8
C = D // 2             # 512 channels per core (8 heads)
NP = S // KP           # 256 pooled positions
P = 128
NK = D // P            # 8 contraction tiles
NCT = C // P           # 4 channel tiles (2 heads each)
NSC = S // 512         # 4 seq chunks in phase A
NORM = float(DD) ** -0.25

dt = mybir.dt
AF = mybir.ActivationFunctionType
OP = mybir.AluOpType


def _emit(nc, tc, aps):
    qT, kT, vT = aps["qT"], aps["kT"], aps["vT"]
    wq, wk, wv = aps["wq"], aps["wk"], aps["wv"]
    wc, wup, mask, biasw, bup2, yT = (
        aps["wc"], aps["wup"], aps["mask"], aps["biasw"], aps["bup2"], aps["yT"])

    with ExitStack() as ctx:
        wpool = ctx.enter_context(tc.tile_pool(name="w", bufs=1))
        xpool = ctx.enter_context(tc.tile_pool(name="x", bufs=2))
        spool = ctx.enter_context(tc.tile_pool(name="s", bufs=3))
        rpool = ctx.enter_context(tc.tile_pool(name="r", bufs=3))
        ppool = ctx.enter_context(tc.tile_pool(name="p", bufs=1))
        apool = ctx.enter_context(tc.tile_pool(name="a", bufs=2))
        ypool = ctx.enter_context(tc.tile_pool(name="y", bufs=2))
        psum = ctx.enter_context(tc.tile_pool(name="ps", bufs=8, space="PSUM"))

        # --- persistent constants/weights.
        # DMA issue order is startup-latency critical: biasw (tiny, needed by
        # the first ACT copy), then q's x-tiles interleaved with q's weights
        # so the first matmuls start ASAP; everything else after.
        biasw_sb = wpool.tile([P, NCT, 3, 8], dt.float32, tag="biasw")
        nc.sync.dma_start(biasw_sb[:], biasw.rearrange("p (t j s) -> p t j s", t=NCT, j=3))

        xT_sbs = {}
        def load_xT(nm, x_ap):
            t = xpool.tile([P, NK, S], dt.bfloat16, tag="xT", name=f"xT_{nm}")
            xr = x_ap.rearrange("(k p) (h s) -> p k h s", p=P, h=2)
            for k in range(NK):
                for hh in range(2):
                    nc.sync.dma_start(t[:, k, hh * (S // 2):(hh + 1) * (S // 2)],
                                      xr[:, k, hh, :])
            xT_sbs[nm] = t

        w_sbs = {}
        def load_w(nm, ap):
            t = wpool.tile([P, NK, C], dt.bfloat16, tag=f"w{nm}", name=f"w_{nm}")
            apr = ap.rearrange("(k p) c -> p k c", p=P)
            for k in range(NK):
                nc.sync.dma_start(t[:, k, :], apr[:, k, :])
            w_sbs[nm] = t

        # q inputs first (interleaved x/w per k-tile), then the rest
        t_x = xpool.tile([P, NK, S], dt.bfloat16, tag="xT", name="xT_q")
        t_w = wpool.tile([P, NK, C], dt.bfloat16, tag="wq", name="w_q")
        xqr = qT.rearrange("(k p) (h s) -> p k h s", p=P, h=2)
        wqr = wq.rearrange("(k p) c -> p k c", p=P)
        for k in range(NK):
            nc.sync.dma_start(t_w[:, k, :], wqr[:, k, :])
            for hh in range(2):
                nc.sync.dma_start(t_x[:, k, hh * (S // 2):(hh + 1) * (S // 2)],
                                  xqr[:, k, hh, :])
        xT_sbs["q"] = t_x
        w_sbs["q"] = t_w
        load_w("k", wk)
        load_w("v", wv)

        wup_sb = wpool.tile([DD, DD], dt.bfloat16, tag="wup")
        nc.sync.dma_start(wup_sb[:], wup[:])
        mask_sb = wpool.tile([P, P], dt.bfloat16, tag="mask")
        nc.sync.dma_start(mask_sb[:], mask[:])
        bup2_sb = wpool.tile([P, 1], dt.float32, tag="bup2")
        nc.sync.dma_start(bup2_sb[:], bup2[:])
        wc_sb = wpool.tile([P, NCT, D], dt.bfloat16, tag="wc")
        wcr = wc.rearrange("(t p) d -> p t d", p=P)
        for t_ in range(NCT):
            nc.sync.dma_start(wc_sb[:, t_, :], wcr[:, t_, :])
        ones_sb = wpool.tile([P, 1], dt.bfloat16, tag="ones")
        nc.vector.memset(ones_sb[:], 1.0)
        ident_sb = wpool.tile([P, P], dt.bfloat16, tag="ident")
        make_identity(nc, ident_sb[:])

        def BW(ct, pj, col):
            return biasw_sb[:, ct, pj, col:col + 1]

        # 3 rotating conv/pool staging buffers; zero pads written once
        xs_tiles = [wpool.tile([P, KP + 1 + S], dt.bfloat16, tag=f"xs{i}",
                               name=f"xs{i}") for i in range(3)]
        for t in xs_tiles:
            nc.vector.memset(t[:, 0:KP + 1], 0.0)

        # --- phase A: projections + causal depthwise conv + causal avg pool.
        # conv taps folded into ONE 8-wide pooled sum (ps2) plus strided
        # edge corrections:
        #   pooled = A*ps2 - B*x[8i] - C*x[8i-1] + B*x[8i-8] + C*x[8i-9] + bcv
        # with A=(w0+w1+w2)/8, B=(w0+w1)/8, C=w0/8 per channel.
        pooled = {}
        for pj, (nm, x_ap) in enumerate((("q", qT), ("k", kT), ("v", vT))):
            if nm not in xT_sbs:
                load_xT(nm, x_ap)
            xT_sb = xT_sbs[nm]
            w_sb = w_sbs[nm]
            pl = ppool.tile([P, NCT, NP], dt.bfloat16, tag=f"pool_{nm}")
            pooled[nm] = pl
            for ct in range(NCT):
                xs = xs_tiles[(pj * NCT + ct) % 3]
                for sc in range(NSC):
                    ps = psum.tile([P, 512], dt.float32, tag="ps")
                    for k in range(NK):
                        nc.tensor.matmul(
                            ps[:], w_sb[:, k, ct * P:(ct + 1) * P],
                            xT_sb[:, k, sc * 512:(sc + 1) * 512],
                            start=(k == 0), stop=(k == NK - 1))
                    nc.scalar.activation(
                        xs[:, KP + 1 + sc * 512: KP + 1 + (sc + 1) * 512], ps[:],
                        AF.Identity, bias=BW(ct, pj, 5), scale=1.0)

                def col(off):  # [256] strided-by-8 view starting at buffer col off
                    return xs[:, off:off + S].rearrange("p (n w) -> p n w", w=KP)[:, :, 0]

                r = rpool.tile([P, NP], dt.float32, tag="ps2")
                nc.vector.tensor_reduce(
                    r[:], xs[:, 2:2 + S].rearrange("p (n w) -> p n w", w=KP),
                    axis=mybir.AxisListType.X, op=OP.add)
                tmp = rpool.tile([P, NP], dt.float32, tag="tmpc")
                nc.vector.tensor_scalar(
                    tmp[:], r[:], BW(ct, pj, 0), BW(ct, pj, 6),
                    op0=OP.mult, op1=OP.add)
                for coli, xoff in ((1, KP + 1), (2, KP), (3, 1)):
                    nc.vector.scalar_tensor_tensor(
                        tmp[:], col(xoff), BW(ct, pj, coli), tmp[:],
                        op0=OP.mult, op1=OP.add)
                nc.vector.scalar_tensor_tensor(
                    pl[:, ct, :], col(0), BW(ct, pj, 4), tmp[:],
                    op0=OP.mult, op1=OP.add)
                # first pooled window only sees conv output 0: fix its bias
                nc.vector.tensor_scalar_add(
                    pl[:, ct, 0:1], pl[:, ct, 0:1], BW(ct, pj, 7))

        # --- phase B prep: vp into [m, c] layout via PE transpose ---
        vpm = [ppool.tile([P, NCT, P], dt.bfloat16, tag=f"vpm{mb}", name=f"vpm{mb}")
               for mb in range(2)]
        for ct in range(NCT):
            for mb in range(2):
                pst = psum.tile([P, P], dt.bfloat16, tag="ps")
                nc.tensor.transpose(
                    pst[:], pooled["v"][:, ct, mb * P:(mb + 1) * P], ident_sb[:])
                nc.vector.tensor_copy(vpm[mb][:, ct, :], pst[:])

        # --- phase B: pooled causal attention (transposed layout), emitted in
        # stages across all 8 heads so independent heads pipeline through
        # PE/ACT/DVE/POOL instead of serializing per head.
        merged = ppool.tile([P, NCT, NP], dt.bfloat16, tag="merged")
        hd = [dict() for _ in range(H // 2)]
        for h in range(H // 2):
            ct, half = h // 2, h % 2
            rows = slice(DD * half, DD * half + DD)
            hd[h]["ct"], hd[h]["rows"] = ct, rows
            qp_h = pooled["q"][rows, ct, :]
            kp_h = pooled["k"][rows, ct, :]
            # E_T[m, n] = exp(qp[n] . kp[m]); block (m1, n0) fully masked -> skipped
            psS0 = psum.tile([P, NP], dt.float32, tag="ps", name=f"psS0_{h}")
            nc.tensor.matmul(psS0[:], kp_h[:, 0:P], qp_h[:, :], start=True, stop=True)
            psS1 = psum.tile([P, P], dt.float32, tag="ps", name=f"psS1_{h}")
            nc.tensor.matmul(psS1[:], kp_h[:, P:NP], qp_h[:, P:NP], start=True, stop=True)
            E0 = apool.tile([P, NP], dt.bfloat16, tag=f"E0_{h}", name=f"E0_{h}")
            nc.scalar.activation(E0[:], psS0[:], AF.Exp)
            E1 = apool.tile([P, P], dt.bfloat16, tag=f"E1_{h}", name=f"E1_{h}")
            nc.scalar.activation(E1[:], psS1[:], AF.Exp)
            nc.vector.tensor_mul(E0[:, 0:P], E0[:, 0:P], mask_sb[:])
            nc.vector.tensor_mul(E1[:], E1[:], mask_sb[:])
            hd[h]["E0"], hd[h]["E1"] = E0, E1
        for h in range(H // 2):
            E0, E1 = hd[h]["E0"], hd[h]["E1"]
            # softmax denominator: column sums of E_T via ones-matmul
            psSum = psum.tile([1, NP], dt.float32, tag="ps", name=f"psSum_{h}")
            nc.tensor.matmul(psSum[:, :], ones_sb[:], E0[:], start=True, stop=False)
            nc.tensor.matmul(psSum[:, P:NP], ones_sb[:], E1[:], start=False, stop=True)
            recip = apool.tile([1, NP], dt.float32, tag=f"recip_{h}", name=f"recip_{h}")
            nc.vector.reciprocal(recip[:], psSum[:])
            rb = apool.tile([DD, NP], dt.float32, tag=f"rb_{h}", name=f"rb_{h}")
            nc.gpsimd.partition_broadcast(rb[:], recip[:])
            hd[h]["rb"] = rb
        for h in range(H // 2):
            ct, rows = hd[h]["ct"], hd[h]["rows"]
            E0, E1, rb = hd[h]["E0"], hd[h]["E1"], hd[h]["rb"]
            # unnormalized out_T[dd, n] = sum_m vp[m, dd] E_T[m, n]
            psU = psum.tile([DD, NP], dt.float32, tag="ps", name=f"psU_{h}")
            nc.tensor.matmul(psU[:], vpm[0][:, ct, rows], E0[:], start=True, stop=False)
            nc.tensor.matmul(psU[:, P:NP], vpm[1][:, ct, rows], E1[:], start=False, stop=True)
            outT = apool.tile([DD, NP], dt.bfloat16, tag=f"outT_{h}", name=f"outT_{h}")
            nc.vector.tensor_mul(outT[:], psU[:], rb[:])
            # shared up-projection: up2_T = Wup.T @ out_T + bup
            psP = psum.tile([DD, NP], dt.float32, tag="ps", name=f"psP_{h}")
            nc.tensor.matmul(psP[:], wup_sb[:], outT[:], start=True, stop=True)
            nc.scalar.activation(
                merged[rows, ct, :], psP[:], AF.Identity,
                bias=bup2_sb[rows, :], scale=1.0)

        # --- phase C: yT = Wc_half.T-partial @ merged ---
        for dti in range(D // P):
            psY = psum.tile([P, NP], dt.float32, tag="ps")
            for ct in range(NCT):
                nc.tensor.matmul(
                    psY[:], wc_sb[:, ct, dti * P:(dti + 1) * P], merged[:, ct, :],
                    start=(ct == 0), stop=(ct == NCT - 1))
            ysb = ypool.tile([P, NP], dt.float32, tag="y")
            nc.scalar.copy(ysb[:], psY[:])
            nc.sync.dma_start(yT[dti * P:(dti + 1) * P, :], ysb[:])


def build():
    nc = bacc.Bacc("TRN2", target_bir_lowering=False, debug=False,
                   num_devices=N_CORES)
    aps = {}
    for nm in ("qT", "kT", "vT"):
        aps[nm] = nc.dram_tensor(nm, [D, S], dt.bfloat16, kind="ExternalInput").ap()
    for nm in ("wq", "wk", "wv"):
        aps[nm] = nc.dram_tensor(nm, [D, C], dt.bfloat16, kind="ExternalInput").ap()
    aps["wc"] = nc.dram_tensor("wc", [C, D], dt.bfloat16, kind="ExternalInput").ap()
    aps["wup"] = nc.dram_tensor("wup", [DD, DD], dt.bfloat16, kind="ExternalInput").ap()
    aps["mask"] = nc.dram_tensor("mask", [P, P], dt.bfloat16, kind="ExternalInput").ap()
    aps["biasw"] = nc.dram_tensor("biasw", [P, NCT * 3 * 8], dt.float32,
                                  kind="ExternalInput").ap()
    aps["bup2"] = nc.dram_tensor("bup2", [P, 1], dt.float32, kind="ExternalInput").ap()
    aps["yT"] = nc.dram_tensor("yT", [D, NP], dt.float32, kind="ExternalOutput").ap()
    with tile.TileContext(nc) as tc:
        _emit(nc, tc, aps)
    nc.compile()
    return nc


_BUILT = None


def _get_built():
    global _BUILT
    if _BUILT is None:
        _BUILT = build()
    return _BUILT


def make_in_maps(q, k, v, Wq, bq, Wk, bk, Wv, bv, Wup, bup, Wc, bc,
                 wcq, bcq, wck, bck, wcv, bcv):
    bf = ml_dtypes.bfloat16
    q, k, v = (np.asarray(x, np.float32) for x in (q, k, v))
    mask_np = np.triu(np.ones((P, P), np.float32)).astype(bf)
    in_maps = []
    for core in range(N_CORES):
        b, half = core // 2, core % 2
        cs = slice(half * C, half * C + C)
        biasw = np.zeros((P, NCT, 3, 8), np.float32)
        for ct in range(NCT):
            ch = slice(half * C + ct * P, half * C + (ct + 1) * P)
            for pj, (cw, cb, db, scale) in enumerate((
                    (wcq, bcq, bq, NORM), (wck, bck, bk, NORM), (wcv, bcv, bv, 1.0))):
                w0, w1, w2 = (np.asarray(cw, np.float32)[:, ch] / KP)
                bconv = np.asarray(cb, np.float32)[ch]
                biasw[:, ct, pj, 0] = w0 + w1 + w2          # A
                biasw[:, ct, pj, 1] = -(w0 + w1)            # -B
                biasw[:, ct, pj, 2] = -w0                   # -C
                biasw[:, ct, pj, 3] = w0 + w1               # +B
                biasw[:, ct, pj, 4] = w0                    # +C
                biasw[:, ct, pj, 5] = np.asarray(db, np.float32)[ch] * scale
                biasw[:, ct, pj, 6] = bconv
                biasw[:, ct, pj, 7] = -(KP - 1) / KP * bconv
        in_maps.append({
            "qT": np.ascontiguousarray(q[b].T).astype(bf),
            "kT": np.ascontiguousarray(k[b].T).astype(bf),
            "vT": np.ascontiguousarray(v[b].T).astype(bf),
            "wq": (np.asarray(Wq, np.float32)[:, cs] * NORM).astype(bf),
            "wk": (np.asarray(Wk, np.float32)[:, cs] * NORM).astype(bf),
            "wv": np.asarray(Wv, np.float32)[:, cs].astype(bf),
            "wc": np.asarray(Wc, np.float32)[cs, :].astype(bf),
            "wup": np.asarray(Wup, np.float32).astype(bf),
            "mask": mask_np,
            "biasw": biasw.reshape(P, NCT * 3 * 8),
            "bup2": np.tile(np.asarray(bup, np.float32), 2).reshape(P, 1),
        })
    return in_maps


def gather(results, bc):
    out = np.empty((B, S, D), np.float32)
    for b in range(B):
        y = results[2 * b]["yT"] + results[2 * b + 1]["yT"]   # [D, NP]
        out[b] = np.repeat(y.T, KP, axis=0) + np.asarray(bc, np.float32)[None, :]
    return out


def kernel(q, k, v, Wq, bq, Wk, bk, Wv, bv, Wup, bup, Wc, bc,
           wcq, bcq, wck, bck, wcv, bcv):
    nc = _get_built()
    in_maps = make_in_maps(q, k, v, Wq, bq, Wk, bk, Wv, bv, Wup, bup, Wc, bc,
                           wcq, bcq, wck, bck, wcv, bcv)
    res = run_bass_kernel_spmd(nc, in_maps, core_ids=list(range(N_CORES)),
                               trace=False)
    return gather(res.results, bc)


# revision 16
# speedup vs baseline: 1.4744x; 1.0665x over previous
"""Trainium2 Bass kernel for nn_MultiHeadAttention_50534585205084 (sparse pooled attention).

Sharding (8 cores): batch (4) x head-half (2). Core c handles batch c//2's
heads [8*(c%2), 8*(c%2)+8) via column-sharded Wq/Wk/Wv and row-sharded Wc.
Each core emits a PARTIAL final projection yT [1024, 256] (pooled rows,
transposed); the host sums the two halves per batch, upsamples rows 8x
(the reference's repeat+crop makes the final output row-periodic with
period KP=8: every op after the pooled attention is position-wise), and
adds bc.

On-chip math (per core), all matmuls bf16 with fp32 PSUM accumulation:
  phase A: for each of q/k/v: xT[1024,2048] @ W -> channel-major conv input
           [512 ch, 2048 seq]; causal depthwise conv (DK=3) fused with causal
           avg-pool (KP=8) as 3 shifted grouped-sum reductions combined with
           per-channel weights (pool's 1/KP and the DD**-0.25 norm are folded
           into host-side weights); all dense/conv biases folded in exactly
           (incl. the i=0 partial-window correction).
  phase B: per head: transposed logits E_T[m,n]=exp(qp.kp) (no max-sub needed:
           |logits|<<1 by construction), causal mask as elementwise 0/1
           multiply on the two diagonal blocks (the all-masked block is
           skipped), softmax denominator via ones-matmul, unnormalized
           out_T = vp_m @ E_T, normalized with a partition-broadcast
           reciprocal, then the shared head up-projection Wup.
  phase C: merged [512, 256] @ row-shard of Wc -> yT [1024, 256].
"""
import sys
sys.path.insert(0, '/opt/trn_rl_repo')

from contextlib import ExitStack

import numpy as np
import ml_dtypes

import concourse.bass as bass
import concourse.mybir as mybir
import concourse.tile as tile
from concourse import bacc
from concourse.bass_utils import run_bass_kernel_spmd
from concourse.masks import make_identity

B, S, D, H, KP, DK = 4, 2048, 1024, 16, 8, 3
DD = D // H            # 64 head dim
N_CORES = 8
C = D // 2             # 512 channels per core (8 heads)
NP = S // KP           # 256 pooled positions
P = 128
NK = D // P            # 8 contraction tiles
NCT = C // P           # 4 channel tiles (2 heads each)
NSC = S // 512         # 4 seq chunks in phase A
NORM = float(DD) ** -0.25

dt = mybir.dt
AF = mybir.ActivationFunctionType
OP = mybir.AluOpType


def _emit(nc, tc, aps):
    qT, kT, vT = aps["qT"], aps["kT"], aps["vT"]
    wq, wk, wv = aps["wq"], aps["wk"], aps["wv"]
    wc, wup, mask, biasw, bup2, yT = (
        aps["wc"], aps["wup"], aps["mask"], aps["biasw"], aps["bup2"], aps["yT"])

    with ExitStack() as ctx:
        wpool = ctx.enter_context(tc.tile_pool(name="w", bufs=1))
        xpool = ctx.enter_context(tc.tile_pool(name="x", bufs=2))
        spool = ctx.enter_context(tc.tile_pool(name="s", bufs=3))
        rpool = ctx.enter_context(tc.tile_pool(name="r", bufs=3))
        ppool = ctx.enter_context(tc.tile_pool(name="p", bufs=1))
        apool = ctx.enter_context(tc.tile_pool(name="a", bufs=2))
        ypool = ctx.enter_context(tc.tile_pool(name="y", bufs=2))
        psum = ctx.enter_context(tc.tile_pool(name="ps", bufs=8, space="PSUM"))

        # --- persistent constants/weights.
        # DMA issue order is startup-latency critical: biasw (tiny, needed by
        # the first ACT copy), then q's x-tiles interleaved with q's weights
        # so the first matmuls start ASAP; everything else after.
        biasw_sb = wpool.tile([P, NCT, 3, 8], dt.float32, tag="biasw")
        nc.sync.dma_start(biasw_sb[:], biasw.rearrange("p (t j s) -> p t j s", t=NCT, j=3))

        xT_sbs = {}
        def load_xT(nm, x_ap):
            t = xpool.tile([P, NK, S], dt.bfloat16, tag="xT", name=f"xT_{nm}")
            xr = x_ap.rearrange("(k p) (h s) -> p k h s", p=P, h=2)
            for k in range(NK):
                for hh in range(2):
                    nc.sync.dma_start(t[:, k, hh * (S // 2):(hh + 1) * (S // 2)],
                                      xr[:, k, hh, :])
            xT_sbs[nm] = t

        w_sbs = {}
        def load_w(nm, ap):
            t = wpool.tile([P, NK, C], dt.bfloat16, tag=f"w{nm}", name=f"w_{nm}")
            apr = ap.rearrange("(k p) c -> p k c", p=P)
            for k in range(NK):
                nc.sync.dma_start(t[:, k, :], apr[:, k, :])
            w_sbs[nm] = t

        # first projection's inputs first (interleaved x/w per k-tile), then the rest
        t_x = xpool.tile([P, NK, S], dt.bfloat16, tag="xT", name="xT_v")
        t_w = wpool.tile([P, NK, C], dt.bfloat16, tag="wv", name="w_v")
        xvr = vT.rearrange("(k p) (h s) -> p k h s", p=P, h=2)
        wvr = wv.rearrange("(k p) c -> p k c", p=P)
        for k in range(NK):
            nc.sync.dma_start(t_w[:, k, :], wvr[:, k, :])
            for hh in range(2):
                nc.sync.dma_start(t_x[:, k, hh * (S // 2):(hh + 1) * (S // 2)],
                                  xvr[:, k, hh, :])
        xT_sbs["v"] = t_x
        w_sbs["v"] = t_w
        load_w("k", wk)
        load_w("q", wq)

        wup_sb = wpool.tile([DD, DD], dt.bfloat16, tag="wup")
        nc.sync.dma_start(wup_sb[:], wup[:])
        mask_sb = wpool.tile([P, P], dt.bfloat16, tag="mask")
        nc.sync.dma_start(mask_sb[:], mask[:])
        bup2_sb = wpool.tile([P, 1], dt.float32, tag="bup2")
        nc.sync.dma_start(bup2_sb[:], bup2[:])
        wc_sb = wpool.tile([P, NCT, D], dt.bfloat16, tag="wc")
        wcr = wc.rearrange("(t p) d -> p t d", p=P)
        for t_ in range(NCT):
            nc.sync.dma_start(wc_sb[:, t_, :], wcr[:, t_, :])
        ones_sb = wpool.tile([P, 1], dt.bfloat16, tag="ones")
        nc.vector.memset(ones_sb[:], 1.0)
        ident_sb = wpool.tile([P, P], dt.bfloat16, tag="ident")
        make_identity(nc, ident_sb[:])

        def BW(ct, pj, col):
            return biasw_sb[:, ct, pj, col:col + 1]

        # 3 rotating conv/pool staging buffers; zero pads written once
        xs_tiles = [wpool.tile([P, KP + 1 + S], dt.bfloat16, tag=f"xs{i}",
                               name=f"xs{i}") for i in range(3)]
        for t in xs_tiles:
            nc.vector.memset(t[:, 0:KP + 1], 0.0)

        # --- phase A: projections + causal depthwise conv + causal avg pool.
        # conv taps folded into ONE 8-wide pooled sum (ps2) plus strided
        # edge corrections:
        #   pooled = A*ps2 - B*x[8i] - C*x[8i-1] + B*x[8i-8] + C*x[8i-9] + bcv
        # with A=(w0+w1+w2)/8, B=(w0+w1)/8, C=w0/8 per channel.
        pooled = {}
        for pji, (nm, x_ap) in enumerate((("v", vT), ("k", kT), ("q", qT))):
            pj = {"q": 0, "k": 1, "v": 2}[nm]   # biasw host-layout index
            if nm not in xT_sbs:
                load_xT(nm, x_ap)
            xT_sb = xT_sbs[nm]
            w_sb = w_sbs[nm]
            pl = ppool.tile([P, NCT, NP], dt.bfloat16, tag=f"pool_{nm}")
            pooled[nm] = pl
            for ct in range(NCT):
                xs = xs_tiles[(pji * NCT + ct) % 3]
                for sc in range(NSC):
                    ps = psum.tile([P, 512], dt.float32, tag="ps")
                    for k in range(NK):
                        nc.tensor.matmul(
                            ps[:], w_sb[:, k, ct * P:(ct + 1) * P],
                            xT_sb[:, k, sc * 512:(sc + 1) * 512],
                            start=(k == 0), stop=(k == NK - 1))
                    nc.scalar.activation(
                        xs[:, KP + 1 + sc * 512: KP + 1 + (sc + 1) * 512], ps[:],
                        AF.Identity, bias=BW(ct, pj, 5), scale=1.0)

                def col(off):  # [256] strided-by-8 view starting at buffer col off
                    return xs[:, off:off + S].rearrange("p (n w) -> p n w", w=KP)[:, :, 0]

                r = rpool.tile([P, NP], dt.float32, tag="ps2")
                nc.vector.tensor_reduce(
                    r[:], xs[:, 2:2 + S].rearrange("p (n w) -> p n w", w=KP),
                    axis=mybir.AxisListType.X, op=OP.add)
                tmp = rpool.tile([P, NP], dt.float32, tag="tmpc")
                nc.vector.tensor_scalar(
                    tmp[:], r[:], BW(ct, pj, 0), BW(ct, pj, 6),
                    op0=OP.mult, op1=OP.add)
                for coli, xoff in ((1, KP + 1), (2, KP), (3, 1)):
                    nc.vector.scalar_tensor_tensor(
                        tmp[:], col(xoff), BW(ct, pj, coli), tmp[:],
                        op0=OP.mult, op1=OP.add)
                nc.vector.scalar_tensor_tensor(
                    pl[:, ct, :], col(0), BW(ct, pj, 4), tmp[:],
                    op0=OP.mult, op1=OP.add)
                # first pooled window only sees conv output 0: fix its bias
                nc.vector.tensor_scalar_add(
                    pl[:, ct, 0:1], pl[:, ct, 0:1], BW(ct, pj, 7))

        # --- phase B prep: vp into [m, c] layout via PE transpose ---
        vpm = [ppool.tile([P, NCT, P], dt.bfloat16, tag=f"vpm{mb}", name=f"vpm{mb}")
               for mb in range(2)]
        for ct in range(NCT):
            for mb in range(2):
                pst = psum.tile([P, P], dt.bfloat16, tag="ps")
                nc.tensor.transpose(
                    pst[:], pooled["v"][:, ct, mb * P:(mb + 1) * P], ident_sb[:])
                nc.vector.tensor_copy(vpm[mb][:, ct, :], pst[:])

        # --- phase B: pooled causal attention (transposed layout), emitted in
        # stages across all 8 heads so independent heads pipeline through
        # PE/ACT/DVE/POOL instead of serializing per head.
        merged = ppool.tile([P, NCT, NP], dt.bfloat16, tag="merged")
        hd = [dict() for _ in range(H // 2)]
        for h in range(H // 2):
            ct, half = h // 2, h % 2
            rows = slice(DD * half, DD * half + DD)
            hd[h]["ct"], hd[h]["rows"] = ct, rows
            qp_h = pooled["q"][rows, ct, :]
            kp_h = pooled["k"][rows, ct, :]
            # E_T[m, n] = exp(qp[n] . kp[m]); block (m1, n0) fully masked -> skipped
            psS0 = psum.tile([P, NP], dt.float32, tag="ps", name=f"psS0_{h}")
            nc.tensor.matmul(psS0[:], kp_h[:, 0:P], qp_h[:, :], start=True, stop=True)
            psS1 = psum.tile([P, P], dt.float32, tag="ps", name=f"psS1_{h}")
            nc.tensor.matmul(psS1[:], kp_h[:, P:NP], qp_h[:, P:NP], start=True, stop=True)
            E0 = apool.tile([P, NP], dt.bfloat16, tag=f"E0_{h}", name=f"E0_{h}")
            nc.scalar.activation(E0[:], psS0[:], AF.Exp)
            E1 = apool.tile([P, P], dt.bfloat16, tag=f"E1_{h}", name=f"E1_{h}")
            nc.scalar.activation(E1[:], psS1[:], AF.Exp)
            nc.vector.tensor_mul(E0[:, 0:P], E0[:, 0:P], mask_sb[:])
            nc.vector.tensor_mul(E1[:], E1[:], mask_sb[:])
            hd[h]["E0"], hd[h]["E1"] = E0, E1
        for h in range(H // 2):
            E0, E1 = hd[h]["E0"], hd[h]["E1"]
            # softmax denominator: column sums of E_T via ones-matmul
            psSum = psum.tile([1, NP], dt.float32, tag="ps", name=f"psSum_{h}")
            nc.tensor.matmul(psSum[:, :], ones_sb[:], E0[:], start=True, stop=False)
            nc.tensor.matmul(psSum[:, P:NP], ones_sb[:], E1[:], start=False, stop=True)
            recip = apool.tile([1, NP], dt.float32, tag=f"recip_{h}", name=f"recip_{h}")
            nc.vector.reciprocal(recip[:], psSum[:])
            rb = apool.tile([DD, NP], dt.float32, tag=f"rb_{h}", name=f"rb_{h}")
            nc.gpsimd.partition_broadcast(rb[:], recip[:])
            hd[h]["rb"] = rb
        for h in range(H // 2):
            ct, rows = hd[h]["ct"], hd[h]["rows"]
            E0, E1, rb = hd[h]["E0"], hd[h]["E1"], hd[h]["rb"]
            # unnormalized out_T[dd, n] = sum_m vp[m, dd] E_T[m, n]
            psU = psum.tile([DD, NP], dt.float32, tag="ps", name=f"psU_{h}")
            nc.tensor.matmul(psU[:], vpm[0][:, ct, rows], E0[:], start=True, stop=False)
            nc.tensor.matmul(psU[:, P:NP], vpm[1][:, ct, rows], E1[:], start=False, stop=True)
            outT = apool.tile([DD, NP], dt.bfloat16, tag=f"outT_{h}", name=f"outT_{h}")
            nc.vector.tensor_mul(outT[:], psU[:], rb[:])
            hd[h]["outT"] = outT
        for h in range(H // 2):
            ct, rows = hd[h]["ct"], hd[h]["rows"]
            # shared up-projection: up2_T = Wup.T @ out_T + bup
            psP = psum.tile([DD, NP], dt.float32, tag="ps", name=f"psP_{h}")
            nc.tensor.matmul(psP[:], wup_sb[:], hd[h]["outT"][:], start=True, stop=True)
            nc.scalar.activation(
                merged[rows, ct, :], psP[:], AF.Identity,
                bias=bup2_sb[rows, :], scale=1.0)

        # --- phase C: yT = Wc_half.T-partial @ merged ---
        for dti in range(D // P):
            psY = psum.tile([P, NP], dt.float32, tag="ps")
            for ct in range(NCT):
                nc.tensor.matmul(
                    psY[:], wc_sb[:, ct, dti * P:(dti + 1) * P], merged[:, ct, :],
                    start=(ct == 0), stop=(ct == NCT - 1))
            ysb = ypool.tile([P, NP], dt.float32, tag="y")
            nc.scalar.copy(ysb[:], psY[:])
            nc.sync.dma_start(yT[dti * P:(dti + 1) * P, :], ysb[:])


def build():
    nc = bacc.Bacc("TRN2", target_bir_lowering=False, debug=False,
                   num_devices=N_CORES)
    aps = {}
    for nm in ("qT", "kT", "vT"):
        aps[nm] = nc.dram_tensor(nm, [D, S], dt.bfloat16, kind="ExternalInput").ap()
    for nm in ("wq", "wk", "wv"):
        aps[nm] = nc.dram_tensor(nm, [D, C], dt.bfloat16, kind="ExternalInput").ap()
    aps["wc"] = nc.dram_tensor("wc", [C, D], dt.bfloat16, kind="ExternalInput").ap()
    aps["wup"] = nc.dram_tensor("wup", [DD, DD], dt.bfloat16, kind="ExternalInput").ap()
    aps["mask"] = nc.dram_tensor("mask", [P, P], dt.bfloat16, kind="ExternalInput").ap()
    aps["biasw"] = nc.dram_tensor("biasw", [P, NCT * 3 * 8], dt.float32,
                                  kind="ExternalInput").ap()
    aps["bup2"] = nc.dram_tensor("bup2", [P, 1], dt.float32, kind="ExternalInput").ap()
    aps["yT"] = nc.dram_tensor("yT", [D, NP], dt.float32, kind="ExternalOutput").ap()
    with tile.TileContext(nc) as tc:
        _emit(nc, tc, aps)
    nc.compile()
    return nc


_BUILT = None


def _get_built():
    global _BUILT
    if _BUILT is None:
        _BUILT = build()
    return _BUILT


def make_in_maps(q, k, v, Wq, bq, Wk, bk, Wv, bv, Wup, bup, Wc, bc,
                 wcq, bcq, wck, bck, wcv, bcv):
    bf = ml_dtypes.bfloat16
    q, k, v = (np.asarray(x, np.float32) for x in (q, k, v))
    mask_np = np.triu(np.ones((P, P), np.float32)).astype(bf)
    in_maps = []
    for core in range(N_CORES):
        b, half = core // 2, core % 2
        cs = slice(half * C, half * C + C)
        biasw = np.zeros((P, NCT, 3, 8), np.float32)
        for ct in range(NCT):
            ch = slice(half * C + ct * P, half * C + (ct + 1) * P)
            for pj, (cw, cb, db, scale) in enumerate((
                    (wcq, bcq, bq, NORM), (wck, bck, bk, NORM), (wcv, bcv, bv, 1.0))):
                w0, w1, w2 = (np.asarray(cw, np.float32)[:, ch] / KP)
                bconv = np.asarray(cb, np.float32)[ch]
                biasw[:, ct, pj, 0] = w0 + w1 + w2          # A
                biasw[:, ct, pj, 1] = -(w0 + w1)            # -B
                biasw[:, ct, pj, 2] = -w0                   # -C
                biasw[:, ct, pj, 3] = w0 + w1               # +B
                biasw[:, ct, pj, 4] = w0                    # +C
                biasw[:, ct, pj, 5] = np.asarray(db, np.float32)[ch] * scale
                biasw[:, ct, pj, 6] = bconv
                biasw[:, ct, pj, 7] = -(KP - 1) / KP * bconv
        in_maps.append({
            "qT": np.ascontiguousarray(q[b].T).astype(bf),
            "kT": np.ascontiguousarray(k[b].T).astype(bf),
            "vT": np.ascontiguousarray(v[b].T).astype(bf),
            "wq": (np.asarray(Wq, np.float32)[:, cs] * NORM).astype(bf),
            "wk": (np.asarray(Wk, np.float32)[:, cs] * NORM).astype(bf),
            "wv": np.asarray(Wv, np.float32)[:, cs].astype(bf),
            "wc": np.asarray(Wc, np.float32)[cs, :].astype(bf),
            "wup": np.asarray(Wup, np.float32).astype(bf),
            "mask": mask_np,
            "biasw": biasw.reshape(P, NCT * 3 * 8),
            "bup2": np.tile(np.asarray(bup, np.float32), 2).reshape(P, 1),
        })
    return in_maps


def gather(results, bc):
    out = np.empty((B, S, D), np.float32)
    for b in range(B):
        y = results[2 * b]["yT"] + results[2 * b + 1]["yT"]   # [D, NP]
        out[b] = np.repeat(y.T, KP, axis=0) + np.asarray(bc, np.float32)[None, :]
    return out


def kernel(q, k, v, Wq, bq, Wk, bk, Wv, bv, Wup, bup, Wc, bc,
           wcq, bcq, wck, bck, wcv, bcv):
    nc = _get_built()
    in_maps = make_in_maps(q, k, v, Wq, bq, Wk, bk, Wv, bv, Wup, bup, Wc, bc,
                           wcq, bcq, wck, bck, wcv, bcv)
    res = run_bass_kernel_spmd(nc, in_maps, core_ids=list(range(N_CORES)),
                               trace=False)
    return gather(res.results, bc)


# revision 17
# speedup vs baseline: 1.5450x; 1.0479x over previous
"""Trainium2 Bass kernel for nn_MultiHeadAttention_50534585205084 (sparse pooled attention).

Sharding (8 cores): batch (4) x head-half (2). Core c handles batch c//2's
heads [8*(c%2), 8*(c%2)+8) via column-sharded Wq/Wk/Wv and row-sharded Wc.
Each core emits a PARTIAL final projection yT [1024, 256] (pooled rows,
transposed); the host sums the two halves per batch, upsamples rows 8x
(the reference's repeat+crop makes the final output row-periodic with
period KP=8: every op after the pooled attention is position-wise), and
adds bc.

On-chip math (per core), all matmuls bf16 with fp32 PSUM accumulation:
  phase A: for each of q/k/v: xT[1024,2048] @ W -> channel-major conv input
           [512 ch, 2048 seq]; causal depthwise conv (DK=3) fused with causal
           avg-pool (KP=8) as 3 shifted grouped-sum reductions combined with
           per-channel weights (pool's 1/KP and the DD**-0.25 norm are folded
           into host-side weights); all dense/conv biases folded in exactly
           (incl. the i=0 partial-window correction).
  phase B: per head: transposed logits E_T[m,n]=exp(qp.kp) (no max-sub needed:
           |logits|<<1 by construction), causal mask as elementwise 0/1
           multiply on the two diagonal blocks (the all-masked block is
           skipped), softmax denominator via ones-matmul, unnormalized
           out_T = vp_m @ E_T, normalized with a partition-broadcast
           reciprocal, then the shared head up-projection Wup.
  phase C: merged [512, 256] @ row-shard of Wc -> yT [1024, 256].
"""
import sys
sys.path.insert(0, '/opt/trn_rl_repo')

from contextlib import ExitStack

import numpy as np
import ml_dtypes

import concourse.bass as bass
import concourse.mybir as mybir
import concourse.tile as tile
from concourse import bacc
from concourse.bass_utils import run_bass_kernel_spmd
from concourse.masks import make_identity

B, S, D, H, KP, DK = 4, 2048, 1024, 16, 8, 3
DD = D // H            # 64 head dim
N_CORES = 8
C = D // 2             # 512 channels per core (8 heads)
NP = S // KP           # 256 pooled positions
P = 128
NK = D // P            # 8 contraction tiles
NCT = C // P           # 4 channel tiles (2 heads each)
NSC = S // 512         # 4 seq chunks in phase A
NORM = float(DD) ** -0.25

dt = mybir.dt
AF = mybir.ActivationFunctionType
OP = mybir.AluOpType


def _emit(nc, tc, aps):
    qT, kT, vT = aps["qT"], aps["kT"], aps["vT"]
    wq, wk, wv = aps["wq"], aps["wk"], aps["wv"]
    wc, wup, mask, biasw, bup2, yT = (
        aps["wc"], aps["wup"], aps["mask"], aps["biasw"], aps["bup2"], aps["yT"])

    with ExitStack() as ctx:
        wpool = ctx.enter_context(tc.tile_pool(name="w", bufs=1))
        xpool = ctx.enter_context(tc.tile_pool(name="x", bufs=2))
        spool = ctx.enter_context(tc.tile_pool(name="s", bufs=3))
        rpool = ctx.enter_context(tc.tile_pool(name="r", bufs=3))
        ppool = ctx.enter_context(tc.tile_pool(name="p", bufs=1))
        apool = ctx.enter_context(tc.tile_pool(name="a", bufs=2))
        ypool = ctx.enter_context(tc.tile_pool(name="y", bufs=8))
        psum = ctx.enter_context(tc.tile_pool(name="ps", bufs=8, space="PSUM"))

        # --- persistent constants/weights.
        # DMA issue order is startup-latency critical: biasw (tiny, needed by
        # the first ACT copy), then q's x-tiles interleaved with q's weights
        # so the first matmuls start ASAP; everything else after.
        biasw_sb = wpool.tile([P, NCT, 3, 8], dt.float32, tag="biasw")
        nc.sync.dma_start(biasw_sb[:], biasw.rearrange("p (t j s) -> p t j s", t=NCT, j=3))

        xT_sbs = {}
        def load_xT(nm, x_ap):
            t = xpool.tile([P, NK, S], dt.bfloat16, tag="xT", name=f"xT_{nm}")
            xr = x_ap.rearrange("(k p) (h s) -> p k h s", p=P, h=2)
            for k in range(NK):
                for hh in range(2):
                    nc.sync.dma_start(t[:, k, hh * (S // 2):(hh + 1) * (S // 2)],
                                      xr[:, k, hh, :])
            xT_sbs[nm] = t

        w_sbs = {}
        def load_w(nm, ap):
            t = wpool.tile([P, NK, C], dt.bfloat16, tag=f"w{nm}", name=f"w_{nm}")
            apr = ap.rearrange("(k p) c -> p k c", p=P)
            for k in range(NK):
                nc.sync.dma_start(t[:, k, :], apr[:, k, :])
            w_sbs[nm] = t

        # first projection's inputs first (interleaved x/w per k-tile), then the rest
        t_x = xpool.tile([P, NK, S], dt.bfloat16, tag="xT", name="xT_v")
        t_w = wpool.tile([P, NK, C], dt.bfloat16, tag="wv", name="w_v")
        xvr = vT.rearrange("(k p) (h s) -> p k h s", p=P, h=2)
        wvr = wv.rearrange("(k p) c -> p k c", p=P)
        for k in range(NK):
            nc.sync.dma_start(t_w[:, k, :], wvr[:, k, :])
            for hh in range(2):
                nc.sync.dma_start(t_x[:, k, hh * (S // 2):(hh + 1) * (S // 2)],
                                  xvr[:, k, hh, :])
        xT_sbs["v"] = t_x
        w_sbs["v"] = t_w
        load_w("k", wk)
        load_w("q", wq)

        wup_sb = wpool.tile([DD, DD], dt.bfloat16, tag="wup")
        nc.sync.dma_start(wup_sb[:], wup[:])
        mask_sb = wpool.tile([P, P], dt.bfloat16, tag="mask")
        nc.sync.dma_start(mask_sb[:], mask[:])
        bup2_sb = wpool.tile([P, 1], dt.float32, tag="bup2")
        nc.sync.dma_start(bup2_sb[:], bup2[:])
        wc_sb = wpool.tile([P, NCT, D], dt.bfloat16, tag="wc")
        wcr = wc.rearrange("(t p) d -> p t d", p=P)
        for t_ in range(NCT):
            nc.sync.dma_start(wc_sb[:, t_, :], wcr[:, t_, :])
        ones_sb = wpool.tile([P, 1], dt.bfloat16, tag="ones")
        nc.vector.memset(ones_sb[:], 1.0)
        ident_sb = wpool.tile([P, P], dt.bfloat16, tag="ident")
        make_identity(nc, ident_sb[:])

        def BW(ct, pj, col):
            return biasw_sb[:, ct, pj, col:col + 1]

        # 3 rotating conv/pool staging buffers; zero pads written once
        xs_tiles = [wpool.tile([P, KP + 1 + S], dt.bfloat16, tag=f"xs{i}",
                               name=f"xs{i}") for i in range(3)]
        for t in xs_tiles:
            nc.vector.memset(t[:, 0:KP + 1], 0.0)

        # --- phase A: projections + causal depthwise conv + causal avg pool.
        # conv taps folded into ONE 8-wide pooled sum (ps2) plus strided
        # edge corrections:
        #   pooled = A*ps2 - B*x[8i] - C*x[8i-1] + B*x[8i-8] + C*x[8i-9] + bcv
        # with A=(w0+w1+w2)/8, B=(w0+w1)/8, C=w0/8 per channel.
        pooled = {}
        for pji, (nm, x_ap) in enumerate((("v", vT), ("k", kT), ("q", qT))):
            pj = {"q": 0, "k": 1, "v": 2}[nm]   # biasw host-layout index
            if nm not in xT_sbs:
                load_xT(nm, x_ap)
            xT_sb = xT_sbs[nm]
            w_sb = w_sbs[nm]
            pl = ppool.tile([P, NCT, NP], dt.bfloat16, tag=f"pool_{nm}")
            pooled[nm] = pl
            for ct in range(NCT):
                xs = xs_tiles[(pji * NCT + ct) % 3]
                for sc in range(NSC):
                    ps = psum.tile([P, 512], dt.float32, tag="ps")
                    for k in range(NK):
                        nc.tensor.matmul(
                            ps[:], w_sb[:, k, ct * P:(ct + 1) * P],
                            xT_sb[:, k, sc * 512:(sc + 1) * 512],
                            start=(k == 0), stop=(k == NK - 1))
                    nc.scalar.activation(
                        xs[:, KP + 1 + sc * 512: KP + 1 + (sc + 1) * 512], ps[:],
                        AF.Identity, bias=BW(ct, pj, 5), scale=1.0)

                def col(off):  # [256] strided-by-8 view starting at buffer col off
                    return xs[:, off:off + S].rearrange("p (n w) -> p n w", w=KP)[:, :, 0]

                r = rpool.tile([P, NP], dt.float32, tag="ps2")
                nc.vector.tensor_reduce(
                    r[:], xs[:, 2:2 + S].rearrange("p (n w) -> p n w", w=KP),
                    axis=mybir.AxisListType.X, op=OP.add)
                tmp = rpool.tile([P, NP], dt.float32, tag="tmpc")
                nc.vector.tensor_scalar(
                    tmp[:], r[:], BW(ct, pj, 0), BW(ct, pj, 6),
                    op0=OP.mult, op1=OP.add)
                for coli, xoff in ((1, KP + 1), (2, KP), (3, 1)):
                    nc.vector.scalar_tensor_tensor(
                        tmp[:], col(xoff), BW(ct, pj, coli), tmp[:],
                        op0=OP.mult, op1=OP.add)
                nc.vector.scalar_tensor_tensor(
                    pl[:, ct, :], col(0), BW(ct, pj, 4), tmp[:],
                    op0=OP.mult, op1=OP.add)
                # first pooled window only sees conv output 0: fix its bias
                nc.vector.tensor_scalar_add(
                    pl[:, ct, 0:1], pl[:, ct, 0:1], BW(ct, pj, 7))

        # --- phase B prep: vp into [m, c] layout via PE transpose ---
        vpm = [ppool.tile([P, NCT, P], dt.bfloat16, tag=f"vpm{mb}", name=f"vpm{mb}")
               for mb in range(2)]
        for ct in range(NCT):
            for mb in range(2):
                pst = psum.tile([P, P], dt.bfloat16, tag="ps")
                nc.tensor.transpose(
                    pst[:], pooled["v"][:, ct, mb * P:(mb + 1) * P], ident_sb[:])
                nc.vector.tensor_copy(vpm[mb][:, ct, :], pst[:])

        # --- phase B: pooled causal attention (transposed layout), emitted in
        # stages across all 8 heads so independent heads pipeline through
        # PE/ACT/DVE/POOL instead of serializing per head.
        merged = ppool.tile([P, NCT, NP], dt.bfloat16, tag="merged")
        hd = [dict() for _ in range(H // 2)]
        for h in range(H // 2):
            ct, half = h // 2, h % 2
            rows = slice(DD * half, DD * half + DD)
            hd[h]["ct"], hd[h]["rows"] = ct, rows
            qp_h = pooled["q"][rows, ct, :]
            kp_h = pooled["k"][rows, ct, :]
            # E_T[m, n] = exp(qp[n] . kp[m]); block (m1, n0) fully masked -> skipped
            psS0 = psum.tile([P, NP], dt.float32, tag="ps", name=f"psS0_{h}")
            nc.tensor.matmul(psS0[:], kp_h[:, 0:P], qp_h[:, :], start=True, stop=True)
            psS1 = psum.tile([P, P], dt.float32, tag="ps", name=f"psS1_{h}")
            nc.tensor.matmul(psS1[:], kp_h[:, P:NP], qp_h[:, P:NP], start=True, stop=True)
            E0 = apool.tile([P, NP], dt.bfloat16, tag=f"E0_{h}", name=f"E0_{h}")
            nc.scalar.activation(E0[:], psS0[:], AF.Exp)
            E1 = apool.tile([P, P], dt.bfloat16, tag=f"E1_{h}", name=f"E1_{h}")
            nc.scalar.activation(E1[:], psS1[:], AF.Exp)
            nc.vector.tensor_mul(E0[:, 0:P], E0[:, 0:P], mask_sb[:])
            nc.vector.tensor_mul(E1[:], E1[:], mask_sb[:])
            hd[h]["E0"], hd[h]["E1"] = E0, E1
        for h in range(H // 2):
            E0, E1 = hd[h]["E0"], hd[h]["E1"]
            # softmax denominator: column sums of E_T via ones-matmul
            psSum = psum.tile([1, NP], dt.float32, tag="ps", name=f"psSum_{h}")
            nc.tensor.matmul(psSum[:, :], ones_sb[:], E0[:], start=True, stop=False)
            nc.tensor.matmul(psSum[:, P:NP], ones_sb[:], E1[:], start=False, stop=True)
            recip = apool.tile([1, NP], dt.float32, tag=f"recip_{h}", name=f"recip_{h}")
            nc.vector.reciprocal(recip[:], psSum[:])
            rb = apool.tile([DD, NP], dt.float32, tag=f"rb_{h}", name=f"rb_{h}")
            nc.gpsimd.partition_broadcast(rb[:], recip[:])
            hd[h]["rb"] = rb
        for h in range(H // 2):
            ct, rows = hd[h]["ct"], hd[h]["rows"]
            E0, E1, rb = hd[h]["E0"], hd[h]["E1"], hd[h]["rb"]
            # unnormalized out_T[dd, n] = sum_m vp[m, dd] E_T[m, n]
            psU = psum.tile([DD, NP], dt.float32, tag="ps", name=f"psU_{h}")
            nc.tensor.matmul(psU[:], vpm[0][:, ct, rows], E0[:], start=True, stop=False)
            nc.tensor.matmul(psU[:, P:NP], vpm[1][:, ct, rows], E1[:], start=False, stop=True)
            outT = apool.tile([DD, NP], dt.bfloat16, tag=f"outT_{h}", name=f"outT_{h}")
            nc.vector.tensor_mul(outT[:], psU[:], rb[:])
            hd[h]["outT"] = outT
        for h in range(H // 2):
            ct, rows = hd[h]["ct"], hd[h]["rows"]
            # shared up-projection: up2_T = Wup.T @ out_T + bup
            psP = psum.tile([DD, NP], dt.float32, tag="ps", name=f"psP_{h}")
            nc.tensor.matmul(psP[:], wup_sb[:], hd[h]["outT"][:], start=True, stop=True)
            nc.scalar.activation(
                merged[rows, ct, :], psP[:], AF.Identity,
                bias=bup2_sb[rows, :], scale=1.0)

        # --- phase C: yT = Wc_half.T-partial @ merged ---
        for dti in range(D // P):
            psY = psum.tile([P, NP], dt.float32, tag="ps")
            for ct in range(NCT):
                nc.tensor.matmul(
                    psY[:], wc_sb[:, ct, dti * P:(dti + 1) * P], merged[:, ct, :],
                    start=(ct == 0), stop=(ct == NCT - 1))
            ysb = ypool.tile([P, NP], dt.float32, tag="y")
            nc.scalar.copy(ysb[:], psY[:])
            nc.sync.dma_start(yT[dti * P:(dti + 1) * P, :], ysb[:])


def build():
    nc = bacc.Bacc("TRN2", target_bir_lowering=False, debug=False,
                   num_devices=N_CORES)
    aps = {}
    for nm in ("qT", "kT", "vT"):
        aps[nm] = nc.dram_tensor(nm, [D, S], dt.bfloat16, kind="ExternalInput").ap()
    for nm in ("wq", "wk", "wv"):
        aps[nm] = nc.dram_tensor(nm, [D, C], dt.bfloat16, kind="ExternalInput").ap()
    aps["wc"] = nc.dram_tensor("wc", [C, D], dt.bfloat16, kind="ExternalInput").ap()
    aps["wup"] = nc.dram_tensor("wup", [DD, DD], dt.bfloat16, kind="ExternalInput").ap()
    aps["mask"] = nc.dram_tensor("mask", [P, P], dt.bfloat16, kind="ExternalInput").ap()
    aps["biasw"] = nc.dram_tensor("biasw", [P, NCT * 3 * 8], dt.float32,
                                  kind="ExternalInput").ap()
    aps["bup2"] = nc.dram_tensor("bup2", [P, 1], dt.float32, kind="ExternalInput").ap()
    aps["yT"] = nc.dram_tensor("yT", [D, NP], dt.float32, kind="ExternalOutput").ap()
    with tile.TileContext(nc) as tc:
        _emit(nc, tc, aps)
    nc.compile()
    return nc


_BUILT = None


def _get_built():
    global _BUILT
    if _BUILT is None:
        _BUILT = build()
    return _BUILT


def make_in_maps(q, k, v, Wq, bq, Wk, bk, Wv, bv, Wup, bup, Wc, bc,
                 wcq, bcq, wck, bck, wcv, bcv):
    bf = ml_dtypes.bfloat16
    q, k, v = (np.asarray(x, np.float32) for x in (q, k, v))
    mask_np = np.triu(np.ones((P, P), np.float32)).astype(bf)
    in_maps = []
    for core in range(N_CORES):
        b, half = core // 2, core % 2
        cs = slice(half * C, half * C + C)
        biasw = np.zeros((P, NCT, 3, 8), np.float32)
        for ct in range(NCT):
            ch = slice(half * C + ct * P, half * C + (ct + 1) * P)
            for pj, (cw, cb, db, scale) in enumerate((
                    (wcq, bcq, bq, NORM), (wck, bck, bk, NORM), (wcv, bcv, bv, 1.0))):
                w0, w1, w2 = (np.asarray(cw, np.float32)[:, ch] / KP)
                bconv = np.asarray(cb, np.float32)[ch]
                biasw[:, ct, pj, 0] = w0 + w1 + w2          # A
                biasw[:, ct, pj, 1] = -(w0 + w1)            # -B
                biasw[:, ct, pj, 2] = -w0                   # -C
                biasw[:, ct, pj, 3] = w0 + w1               # +B
                biasw[:, ct, pj, 4] = w0                    # +C
                biasw[:, ct, pj, 5] = np.asarray(db, np.float32)[ch] * scale
                biasw[:, ct, pj, 6] = bconv
                biasw[:, ct, pj, 7] = -(KP - 1) / KP * bconv
        in_maps.append({
            "qT": np.ascontiguousarray(q[b].T).astype(bf),
            "kT": np.ascontiguousarray(k[b].T).astype(bf),
            "vT": np.ascontiguousarray(v[b].T).astype(bf),
            "wq": (np.asarray(Wq, np.float32)[:, cs] * NORM).astype(bf),
            "wk": (np.asarray(Wk, np.float32)[:, cs] * NORM).astype(bf),
            "wv": np.asarray(Wv, np.float32)[:, cs].astype(bf),
            "wc": np.asarray(Wc, np.float32)[cs, :].astype(bf),
            "wup": np.asarray(Wup, np.float32).astype(bf),
            "mask": mask_np,
            "biasw": biasw.reshape(P, NCT * 3 * 8),
            "bup2": np.tile(np.asarray(bup, np.float32), 2).reshape(P, 1),
        })
    return in_maps


def gather(results, bc):
    out = np.empty((B, S, D), np.float32)
    for b in range(B):
        y = results[2 * b]["yT"] + results[2 * b + 1]["yT"]   # [D, NP]
        out[b] = np.repeat(y.T, KP, axis=0) + np.asarray(bc, np.float32)[None, :]
    return out


def kernel(q, k, v, Wq, bq, Wk, bk, Wv, bv, Wup, bup, Wc, bc,
           wcq, bcq, wck, bck, wcv, bcv):
    nc = _get_built()
    in_maps = make_in_maps(q, k, v, Wq, bq, Wk, bk, Wv, bv, Wup, bup, Wc, bc,
                           wcq, bcq, wck, bck, wcv, bcv)
    res = run_bass_kernel_spmd(nc, in_maps, core_ids=list(range(N_CORES)),
                               trace=False)
    return gather(res.results, bc)
